# revision 69
# baseline (speedup 1.0000x reference)
"""Causal self-attention Trainium2 kernel (bf16, software-pipelined).

Problem: B=2, L=2048, D=1024, 16 heads (hd=64), fp32 reference.

Sharding (8 cores): core = (batch b in {0,1}) x (head-group g in {0..3} of 4
heads). Each core:
  - reads x[b]^T in bf16, [qb][c][128,512] chunks contiguous in DRAM so DMA
    packets stay large (fp32 d-chunk loads dribbled at 2KB/packet)
  - QKV projection for its 4 heads (bf16 matmuls, 1 cycle/row at any N,
    fp32 PSUM accumulation)
  - causal attention, software-pipelined per (head-pair t, k-chunk kc):
      S^T[k, q] = K^T(lhsT) x Q^T(rhs), one 128-token k-chunk per step;
      diagonal chunks compute only q >= 128*c_rel (N = 512/384/256/128)
      P^T = exp(S^T) on ACT, one [128, 2, 512] instr per step (both head
      halves share a 2-bank PSUM tile); tril mask via one [128,128] const
      O^T accumulation lags S/exp by TWO steps so the PE (which is nearly
      dead-even with ACT's exp load) never stalls on it; emission order per
      step is [O(i-2), S(i), exp(i)]
      normalize: 1/rowsum = exp(-ln(.)) on ACT (one table set, patched, no
      switches), partition-broadcast on the idle GPSIMD engine, single
      PSUM-operand multiply on DVE
  - output projection in [2 matmul + copy] sub-units assembling [128,1024]
    bf16 tiles, out-DMAs alternating between the SP and SWDGE queues
Host: sums the 4 bf16 head-group partials per batch in fp32, adds out_b.

Scheduling: global PE matmul time and ACT exp time are within ~10% of each
other, so every attention window must carry enough matmul filler to cover
its exp time or the PE idles and HAM re-throttles it to half clock.
KV-lazy: block tb's k/v projections ride inside earlier attention windows
up to the step that first touches them; all outproj units fill attn(3)'s
back half (the only dependency-legal late filler); the final norm runs at
column-half granularity so the tail outproj starts ~2us after the last O.
Startup: x chunk 0 + first wq chunks race over three DMA queues (SP,
SWDGE, ACT ring); the exp activation table is touched at t~1us so its
~2.7us load never stalls the first attention step.

Precision: bf16 inputs/weights/P/V/partials, fp32 PSUM accumulation
-> rel err ~4.5e-3 (harness gate 2e-2).

PSUM budget (8 banks): 2x st [128,2,512] double-buffered (4), 2x O-acc
[65,512] (one head-pair live at a time), 2x qkv/outproj accumulators.
"""
import numpy as np

import concourse.bass as bass
import concourse.mybir as mybir
import concourse.tile as tile
from concourse import bacc
from concourse.bass_utils import run_bass_kernel_spmd

F32 = mybir.dt.float32
F32R = mybir.dt.float32r
BF16 = mybir.dt.bfloat16
AF = mybir.ActivationFunctionType

D_MODEL = 1024
N_HEADS = 16
HD = 64
B = 2
L = 2048                      # tokens per batch
HPC = 4                       # heads per core
DG = HPC * HD                 # 256 dims per core's head group
QB = 512                      # q-block width
N_QB = L // QB                # 4
N_DC = D_MODEL // 128         # 8 d_model chunks
N_TT = L // 128               # 16 token tiles

# q-start offset per diagonal chunk position (c_rel 0..3); N = QB - q0 must
# stay >= 256 for full fp32r rate, so c_rel=3 pads down to q0=256.
_Q0 = (0, 128, 256, 384)


def _patch_act_tables():
    """Make Exp and Ln resolve to the one table set containing both, so the
    per-qblock reciprocal (exp(-ln(x))) never triggers a ~2.7us table switch."""
    from concourse.hw_specs import get_activation_tables
    tabs = get_activation_tables("gen3")
    combined = "natural_log_exp_and_others"
    if combined in tabs:
        for name, fns in tabs.items():
            if name != combined:
                fns.discard(AF.Exp)
                fns.discard(AF.Ln)


def _build():
    _patch_act_tables()
    nc = bacc.Bacc("TRN2", target_bir_lowering=False)

    # x^T in quad-chunk layout [qb][quad][p][ci][q]: each DMA moves a
    # [128, 4, 512] quad with 4KB contiguous per partition (large packets)
    xt = nc.dram_tensor("xt", [N_QB, 2, 128, 4, QB], BF16,
                        kind="ExternalInput")
    wq = nc.dram_tensor("wq", [128, N_DC, DG], BF16, kind="ExternalInput")
    wk = nc.dram_tensor("wk", [128, N_DC, DG], BF16, kind="ExternalInput")
    wv = nc.dram_tensor("wv", [128, N_DC, DG], BF16, kind="ExternalInput")
    wo = nc.dram_tensor("wo", [128, 2, D_MODEL], BF16, kind="ExternalInput")
    bq = nc.dram_tensor("bq", [128, 2], F32, kind="ExternalInput")
    bk = nc.dram_tensor("bk", [128, 2], F32, kind="ExternalInput")
    bv = nc.dram_tensor("bv", [1, DG], BF16, kind="ExternalInput")
    # mask[p, j] = 1 iff j >= 128 + p.  mask[:, 128:] is a [128,128] tril
    # (applied to the 128-q strip containing the diagonal of chunks c_rel
    # 0..2); the full [128,256] handles c_rel=3 whose computed range starts
    # 128 q before its diagonal strip.
    mask = nc.dram_tensor("mask", [128, 128], BF16, kind="ExternalInput")
    # partials summed on host in fp32; bf16 partials cost ~1e-3 extra rel
    # err and halve the 8MB-per-core output traffic (and the tail DMA)
    out = nc.dram_tensor("out", [L, D_MODEL], BF16, kind="ExternalOutput")

    with tile.TileContext(nc) as tc:
        with (
            tc.tile_pool(name="cst", bufs=1) as cst,
            tc.tile_pool(name="xtp", bufs=2) as xtp,
            tc.tile_pool(name="ptp", bufs=5) as ptp,
            tc.tile_pool(name="nrm", bufs=2) as nrm,
            tc.tile_pool(name="osb", bufs=3) as osb,
            tc.tile_pool(name="ps_st", bufs=2, space="PSUM") as ps_st,
            tc.tile_pool(name="ps_ot", bufs=1, space="PSUM") as ps_ot,
            tc.tile_pool(name="ps_acc", bufs=2, space="PSUM") as ps_acc,
        ):
            # ---- constants / weights ----
            wq_sb = cst.tile([128, N_DC, DG], BF16, tag="wq")
            wk_sb = cst.tile([128, N_DC, DG], BF16, tag="wk")
            wv_sb = cst.tile([128, N_DC, DG], BF16, tag="wv")
            wo_sb = cst.tile([128, 2, D_MODEL], BF16, tag="wo")
            bq_sb = cst.tile([128, 2], F32, tag="bq")
            bk_sb = cst.tile([128, 2], F32, tag="bk")
            bv_sb = cst.tile([1, DG], BF16, tag="bv")
            mask_sb = cst.tile([128, 128], BF16, tag="mask")

            # ---- resident activation tensors ----
            qt_sb = [cst.tile([128, L], BF16, tag=f"qt{t}", name=f"qt{t}")
                     for t in range(2)]
            kt_sb = [cst.tile([128, L], BF16, tag=f"kt{t}", name=f"kt{t}")
                     for t in range(2)]
            ot_sb = [cst.tile([128, L], BF16, tag=f"ot{t}", name=f"ot{t}")
                     for t in range(2)]
            v_sb = [cst.tile([128, HPC, HD + 1], BF16, tag=f"v{tt}", name=f"v{tt}")
                    for tt in range(N_TT)]

            # ---- startup DMA: x block 0 + weights over THREE queues.
            # The SP queue (sync) comes alive first (~8.8us) — it carries the
            # first-matmul gate (wq chunk + x chunk). The SWDGE queue
            # (gpsimd) spins up ~3us later; the ACT ring (scalar) is free
            # until the first exp at ~25us. Order follows consumption:
            # q-units -> v-units -> k-units. ----
            xt0 = xtp.tile([128, N_DC, QB], BF16, tag="xt", name="xt0")
            nc.sync.dma_start(wq_sb[:, 0:4], wq[:, 0:4, :])
            nc.scalar.dma_start(xt0[:, 0:2, :], xt[0, 0][:, 0:2, :])
            nc.gpsimd.dma_start(wq_sb[:, 4:], wq[:, 4:, :])
            nc.scalar.dma_start(xt0[:, 2:4, :], xt[0, 0][:, 2:4, :])
            nc.sync.dma_start(xt0[:, 4:, :], xt[0, 1])
            nc.scalar.dma_start(bq_sb, bq[:, :])
            nc.sync.dma_start(wk_sb[:, 0:4], wk[:, 0:4, :])
            nc.scalar.dma_start(wk_sb[:, 4:], wk[:, 4:, :])
            nc.gpsimd.dma_start(wv_sb, wv[:, :, :])
            nc.gpsimd.dma_start(bv_sb, bv[:, :])
            nc.sync.dma_start(bk_sb, bk[:, :])
            nc.gpsimd.dma_start(mask_sb, mask[:, :])

            ones_f = cst.tile([128, HPC], F32, tag="ones_f")
            nc.vector.memset(ones_f, 1.0)
            ones1_f = cst.tile([1, 128], F32, tag="ones1_f")
            nc.vector.memset(ones1_f, 1.0)
            ones1 = cst.tile([1, 128], F32R, tag="ones1")
            nc.vector.tensor_copy(ones1, ones1_f)
            ones1b = cst.tile([1, 128], BF16, tag="ones1b")
            nc.vector.tensor_copy(ones1b, ones1_f)
            # touch Exp right away: the ~2.7us ACT table load lands here at
            # t~1us instead of stalling the first attention step at ~27us
            warm = cst.tile([1, 16], F32, tag="warm")
            nc.scalar.activation(warm, ones1_f[:, 0:16], AF.Exp)

            def load_xt(tb, queues=("sync", "gpsimd")):
                xt_t = xtp.tile([128, N_DC, QB], BF16, tag="xt", name="xt_t")
                for qd in range(2):
                    q = getattr(nc, queues[qd])
                    q.dma_start(xt_t[:, 4 * qd:4 * (qd + 1), :], xt[tb, qd])
                return xt_t

            def qkv_units(tb, xt_t):
                """QKV projection for token block tb: fine sub-units (~4 mm
                each) so the interleaver can pack them between attn steps.
                Order q, v, k matches the startup weight-DMA order."""
                units = []

                def qk_unit(t, w_sb, b_sb, dst):
                    hold = {}

                    def f1():
                        acc = ps_acc.tile([128, QB], F32, tag="mm", name="acc")
                        hold["acc"] = acc
                        for c in range(N_DC // 2):
                            nc.tensor.matmul(
                                acc,
                                w_sb[:, c, 128 * t:128 * (t + 1)],
                                xt_t[:, c, :],
                                start=(c == 0), stop=False,
                            )

                    def f2():
                        acc = hold["acc"]
                        for c in range(N_DC // 2, N_DC):
                            nc.tensor.matmul(
                                acc,
                                w_sb[:, c, 128 * t:128 * (t + 1)],
                                xt_t[:, c, :],
                                start=False, stop=(c == N_DC - 1),
                            )
                        nc.vector.tensor_scalar_add(
                            dst[t][:, tb * QB:(tb + 1) * QB], acc,
                            b_sb[:, t:t + 1],
                        )
                    return [f1, f2]

                def v_unit(j):
                    hold = {}
                    tt = tb * (QB // 128) + j

                    def f1():
                        vps = ps_acc.tile([128, DG], F32, tag="mm", name="vps")
                        hold["acc"] = vps
                        for c in range(N_DC // 2):
                            nc.tensor.matmul(
                                vps,
                                xt_t[:, c, j * 128:(j + 1) * 128],
                                wv_sb[:, c, :],
                                start=(c == 0), stop=False,
                            )

                    def f2():
                        vps = hold["acc"]
                        for c in range(N_DC // 2, N_DC):
                            nc.tensor.matmul(
                                vps,
                                xt_t[:, c, j * 128:(j + 1) * 128],
                                wv_sb[:, c, :],
                                start=False, stop=False,
                            )
                        nc.tensor.matmul(vps, ones1b, bv_sb, start=False,
                                         stop=True)
                        nc.vector.tensor_copy(
                            v_sb[tt][:, :, 0:HD],
                            vps.rearrange("p (h d) -> p h d", h=HPC),
                        )
                        nc.vector.tensor_copy(v_sb[tt][:, :, HD], ones_f)
                    return [f1, f2]

                for t in range(2):
                    units += qk_unit(t, wq_sb, bq_sb, qt_sb)
                kv = []
                for j in range(QB // 128):
                    kv += v_unit(j)
                for t in range(2):
                    kv += qk_unit(t, wk_sb, bk_sb, kt_sb)
                # (q_units, kv_units): attention on block tb needs the
                # q_units for every step, but kv_units only from the step
                # that touches this block's k-chunks — kv can lag.
                return units, kv



            def attn_steps(qb, final_units=(), free_units=()):
                """Attention for q-block qb as pipelined steps.

                Step i emits [O(i-2), norm-if-pair-done, S(i), exp(i)]: the
                O-accumulate lags S by TWO steps, so its exp has two full
                steps (plus interleaved filler) of ACT time — the PE never
                waits on the co-bottlenecked ACT engine.

                final_units (outproj of this qb) are interleaved into the
                last norm at half-block granularity to keep the PE warm
                through the tail.
                """
                n_kc = 4 * (qb + 1)
                seq = [(t, kc) for t in range(2) for kc in range(n_kc)]
                p_hold = {}
                ot_state = {}

                def q0_of(kc):
                    c_rel = kc - 4 * qb
                    return 0 if c_rel < 0 else _Q0[c_rel]

                def emit_s(t, kc):
                    q0 = q0_of(kc)
                    c_rel = kc - 4 * qb
                    # both head-halves packed in one 2-bank tile so a single
                    # exp covers them (halves ACT instruction overhead)
                    st = ps_st.tile([128, 2, QB], F32, tag="st", name="st")
                    p = ptp.tile([128, 2, QB], BF16, tag="pt", name="pt")
                    for hp in range(2):
                        nc.tensor.matmul(
                            st[:, hp, q0:],
                            kt_sb[t][64 * hp:64 * (hp + 1),
                                     kc * 128:(kc + 1) * 128],
                            qt_sb[t][64 * hp:64 * (hp + 1),
                                     qb * QB + q0:(qb + 1) * QB],
                            start=True, stop=True,
                        )
                    nc.scalar.activation(p[:, :, q0:], st[:, :, q0:], AF.Exp)
                    if c_rel >= 0:
                        for hp in range(2):
                            nc.vector.tensor_mul(
                                p[:, hp, q0:q0 + 128],
                                p[:, hp, q0:q0 + 128],
                                mask_sb)
                    p_hold[(t, kc)] = p

                def emit_o(t, kc):
                    if kc == 0:
                        ot_state[t] = [
                            ps_ot.tile([HD + 1, QB], F32, tag=f"otp{hp}",
                                       name=f"otp{hp}") for hp in range(2)]
                    ot_p = ot_state[t]
                    q0 = q0_of(kc)
                    p = p_hold.pop((t, kc))
                    for hp in range(2):
                        nc.tensor.matmul(
                            ot_p[hp][:, q0:],
                            v_sb[kc][:, 2 * t + hp, 0:HD + 1],
                            p[:, hp, q0:],
                            start=(kc == 0), stop=(kc == n_kc - 1),
                        )

                def emit_norm(t):
                    # 1/rowsum = exp(-ln(rowsum)) on ACT (same table set, no
                    # switch); partition-broadcast on the idle GPSIMD engine,
                    # so the final mul needs only one PSUM operand (ot_p).
                    ot_p = ot_state[t]
                    for hp in range(2):
                        lnrs = nrm.tile([1, QB], F32, tag="lnrs",
                                        name="lnrs")
                        nc.scalar.activation(lnrs, ot_p[hp][HD:HD + 1, :],
                                             AF.Ln)
                        recip = nrm.tile([1, QB], F32R, tag="recip",
                                         name="recip")
                        nc.scalar.activation(recip, lnrs, AF.Exp,
                                             scale=-1.0)
                        rbc = nrm.tile([HD, QB], F32R, tag="rbc",
                                       name="rbc")
                        nc.gpsimd.partition_broadcast(rbc, recip)
                        nc.vector.tensor_mul(
                            ot_sb[t][64 * hp:64 * (hp + 1),
                                     qb * QB:(qb + 1) * QB],
                            ot_p[hp][0:HD, :],
                            rbc,
                        )

                def emit_norm_half(t, h):
                    # column-half recip chain: ln -> exp(-x) -> partition
                    # broadcast -> multiply, for q columns [256h, 256h+256)
                    ot_p = ot_state[t]
                    cs = slice(256 * h, 256 * (h + 1))
                    recip = nrm.tile([1, 2, 256], F32R, tag="recip2",
                                     name="recip2")
                    for hp in range(2):
                        lnrs = nrm.tile([1, 256], F32, tag="lnrs",
                                        name="lnrs")
                        nc.scalar.activation(lnrs, ot_p[hp][HD:HD + 1, cs],
                                             AF.Ln)
                        nc.scalar.activation(recip[0:1, hp, :], lnrs,
                                             AF.Exp, scale=-1.0)
                    rbc = nrm.tile([HD, 2, 256], F32R, tag="rbc2",
                                   name="rbc2")
                    nc.gpsimd.partition_broadcast(rbc, recip)
                    for hp in range(2):
                        nc.vector.tensor_mul(
                            ot_sb[t][64 * hp:64 * (hp + 1),
                                     qb * QB + 256 * h:
                                     qb * QB + 256 * (h + 1)],
                            ot_p[hp][0:HD, cs],
                            rbc[:, hp, :],
                        )

                fin = list(final_units)
                nh = len(fin) // 2

                def make_step(i):
                    def f():
                        if i >= 2:
                            pt, pkc = seq[i - 2]
                            emit_o(pt, pkc)
                            if fin and pt == 1:
                                # final q-block, pair 1: the diagonal chunks
                                # n_kc-2/n_kc-1 only touch q >= 256, so the
                                # first column-half of the norm (and the
                                # outproj tiles it feeds) overlaps the last
                                # two attention steps
                                if pkc == n_kc - 3:
                                    for u in free_units:
                                        u()
                                    emit_norm_half(1, 0)
                                    for u in fin[:nh]:
                                        u()
                                elif pkc == n_kc - 1:
                                    emit_norm_half(1, 1)
                                    for u in fin[nh:]:
                                        u()
                            elif pkc == n_kc - 1:
                                emit_norm(pt)
                        if i < len(seq):
                            emit_s(*seq[i])
                    return f

                return [make_step(i) for i in range(len(seq) + 2)]

            def outproj_units(qb):
                """Output projection for q-block qb: 8 sub-units of
                [2 matmul + copy]; each token tile assembles a [128,1024]
                buffer so the out-DMA moves 4KB per partition (large DMA
                packets), queues alternating per tile."""
                units = []

                def op_unit(j, dc, hold):
                    def f():
                        tt = qb * (QB // 128) + j
                        ops = ps_acc.tile([128, 512], F32, tag="mm",
                                          name="ops")
                        for t in range(2):
                            nc.tensor.matmul(
                                ops,
                                ot_sb[t][:, tt * 128:(tt + 1) * 128],
                                wo_sb[:, t, dc * 512:(dc + 1) * 512],
                                start=(t == 0), stop=(t == 1),
                            )
                        if dc == 0:
                            hold["ob"] = osb.tile([128, D_MODEL], BF16,
                                                  tag="ob", name="ob")
                        ob = hold["ob"]
                        nc.vector.tensor_copy(
                            ob[:, dc * 512:(dc + 1) * 512], ops)
                        if dc == 1:
                            # the last q-block's final two tiles go on the
                            # SP queue: the end-of-kernel SWDGE drain is
                            # slow, so gpsimd's last out-DMA must be early
                            even = (j % 2 == 0) if qb < 3 else (j >= 2)
                            q = nc.sync if even else nc.gpsimd
                            q.dma_start(out[tt * 128:(tt + 1) * 128, :], ob)
                    return f

                for j in range(QB // 128):
                    hold = {}
                    for dc in range(2):
                        units.append(op_unit(j, dc, hold))
                return units

            def emit_interleaved(a_units, b_units):
                """Merge two unit lists proportionally (a is the primary
                stream); keeps relative order within each list."""
                na, nb = len(a_units), len(b_units)
                if nb == 0:
                    for u in a_units:
                        u()
                    return
                bi = 0
                for ai, u in enumerate(a_units):
                    u()
                    want = ((ai + 1) * nb) // na
                    while bi < want:
                        b_units[bi]()
                        bi += 1
                while bi < nb:
                    b_units[bi]()
                    bi += 1

            def emit_front_loaded(a_units, front_units, b_units, n_front):
                """front_units distributed over the first n_front steps (they
                must complete by then); b_units spread over the rest."""
                nf = len(front_units)
                fi = 0
                for i, u in enumerate(a_units[:n_front]):
                    u()
                    want = ((i + 1) * nf) // n_front
                    while fi < want:
                        front_units[fi]()
                        fi += 1
                while fi < nf:
                    front_units[fi]()
                    fi += 1
                emit_interleaved(a_units[n_front:], b_units)

            # ---- emission ----
            # Global PE and ACT(exp) load are nearly equal, so filler
            # distribution decides the wall-clock: every attention window
            # must carry enough matmul filler to cover its exp time, else
            # the PE idles and HAM re-throttles it to half clock.
            # KV-lazy: block tb's k/v projections ride INSIDE attention
            # windows (they are only needed once the diagonal approaches),
            # freeing earlier windows' filler for where it is scarce.
            xt1 = load_xt(1, queues=("gpsimd", "sync"))
            nc.gpsimd.dma_start(wo_sb, wo[:, :, :])
            q0u, kv0u = qkv_units(0, xt0)
            for u in q0u + kv0u:
                u()
            q1u, kv1u = qkv_units(1, xt1)
            emit_interleaved(attn_steps(0), q1u + kv1u)
            xt2 = load_xt(2)
            q2u, kv2u = qkv_units(2, xt2)
            emit_interleaved(attn_steps(1), q2u + kv2u[:6])
            xt3 = load_xt(3, queues=("gpsimd", "sync"))
            q3u, kv3u = qkv_units(3, xt3)
            op012 = outproj_units(0) + outproj_units(1) + outproj_units(2)
            emit_front_loaded(attn_steps(2), kv2u[6:], q3u + op012[:2],
                              n_front=8)
            emit_front_loaded(
                attn_steps(3, final_units=outproj_units(3),
                           free_units=op012[-4:]),
                kv3u, op012[2:-4], n_front=12)

    nc.compile()
    return nc


_NC_CACHE = None


def _get_nc():
    global _NC_CACHE
    if _NC_CACHE is None:
        _NC_CACHE = _build()
    return _NC_CACHE


import ml_dtypes

BF16_NP = ml_dtypes.bfloat16


def _sw_w(w):
    """[C*128, M] -> [128, C, M] (SBUF layout, contiguous per partition)."""
    c128, m = w.shape
    return np.ascontiguousarray(
        w.reshape(c128 // 128, 128, m).transpose(1, 0, 2).astype(BF16_NP))


def _make_mask():
    p_ = np.arange(128)[:, None]
    j_ = np.arange(128)[None, :]
    return np.ascontiguousarray((j_ >= p_).astype(BF16_NP))


def kernel(x, qkv_w, qkv_b, out_w, out_b, _trace=False):
    x = np.asarray(x, dtype=np.float32)
    qkv_w = np.asarray(qkv_w, dtype=np.float32)
    qkv_b = np.asarray(qkv_b, dtype=np.float32)
    out_w = np.asarray(out_w, dtype=np.float32)
    out_b = np.asarray(out_b, dtype=np.float32)

    scale = 1.0 / np.sqrt(HD)
    wq_full = qkv_w[0:D_MODEL] * scale          # [1024, 1024]
    wk_full = qkv_w[D_MODEL:2 * D_MODEL]
    wv_full = qkv_w[2 * D_MODEL:3 * D_MODEL]
    bq_full = qkv_b[0:D_MODEL] * scale
    bk_full = qkv_b[D_MODEL:2 * D_MODEL]
    bv_full = qkv_b[2 * D_MODEL:3 * D_MODEL]

    mask = _make_mask()
    in_maps = []
    for core in range(8):
        b, g = core // 4, core % 4
        sl = slice(DG * g, DG * (g + 1))
        # xt: x[b]^T [1024, 2048] -> [qb, quad, p, ci, q] = [4, 2, 128, 4,
        # 512]; each quad is 4KB contiguous per partition (large packets)
        xt_sw = np.ascontiguousarray(
            x[b].T.reshape(2, 4, 128, N_QB, QB).transpose(3, 0, 2, 1, 4)
            .astype(BF16_NP))
        in_maps.append({
            "xt": xt_sw,
            "wq": _sw_w(wq_full[sl].T),
            "wk": _sw_w(wk_full[sl].T),
            "wv": _sw_w(wv_full[sl].T),
            "wo": _sw_w(out_w[:, sl].T),
            "bq": np.ascontiguousarray(bq_full[sl].reshape(2, 128).T),
            "bk": np.ascontiguousarray(bk_full[sl].reshape(2, 128).T),
            "bv": np.ascontiguousarray(bv_full[sl].reshape(1, DG).astype(BF16_NP)),
            "mask": mask,
        })

    nc = _get_nc()
    res = run_bass_kernel_spmd(nc, in_maps, core_ids=list(range(8)),
                               trace=_trace)

    final = np.zeros((B, L, D_MODEL), dtype=np.float32)
    for core in range(8):
        b = core // 4
        final[b] += res.results[core]["out"].astype(np.float32)
    final += out_b[None, None, :]

    kernel.last_results = res
    return final


# revision 70
# speedup vs baseline: 1.0137x; 1.0137x over previous
"""Causal self-attention Trainium2 kernel (bf16, software-pipelined).

Problem: B=2, L=2048, D=1024, 16 heads (hd=64), fp32 reference.

Sharding (8 cores): core = (batch b in {0,1}) x (head-group g in {0..3} of 4
heads). Each core:
  - reads x[b]^T in bf16, [qb][c][128,512] chunks contiguous in DRAM so DMA
    packets stay large (fp32 d-chunk loads dribbled at 2KB/packet)
  - QKV projection for its 4 heads (bf16 matmuls, 1 cycle/row at any N,
    fp32 PSUM accumulation)
  - causal attention, software-pipelined per (head-pair t, k-chunk kc):
      S^T[k, q] = K^T(lhsT) x Q^T(rhs), one 128-token k-chunk per step;
      diagonal chunks compute only q >= 128*c_rel (N = 512/384/256/128)
      P^T = exp(S^T) on ACT, one [128, 2, 512] instr per step (both head
      halves share a 2-bank PSUM tile); tril mask via one [128,128] const
      O^T accumulation lags S/exp by TWO steps so the PE (which is nearly
      dead-even with ACT's exp load) never stalls on it; emission order per
      step is [O(i-2), S(i), exp(i)]
      normalize: 1/rowsum = exp(-ln(.)) on ACT (one table set, patched, no
      switches), partition-broadcast on the idle GPSIMD engine, single
      PSUM-operand multiply on DVE
  - output projection in [2 matmul + copy] sub-units assembling [128,1024]
    bf16 tiles, out-DMAs alternating between the SP and SWDGE queues
Host: sums the 4 bf16 head-group partials per batch in fp32, adds out_b.

Scheduling: global PE matmul time and ACT exp time are within ~10% of each
other, so every attention window must carry enough matmul filler to cover
its exp time or the PE idles and HAM re-throttles it to half clock.
KV-lazy: block tb's k/v projections ride inside earlier attention windows
up to the step that first touches them; all outproj units fill attn(3)'s
back half (the only dependency-legal late filler); the final norm runs at
column-half granularity so the tail outproj starts ~2us after the last O.
Startup: x chunk 0 + first wq chunks race over three DMA queues (SP,
SWDGE, ACT ring); the exp activation table is touched at t~1us so its
~2.7us load never stalls the first attention step.

Precision: bf16 inputs/weights/P/V/partials, fp32 PSUM accumulation
-> rel err ~4.5e-3 (harness gate 2e-2).

PSUM budget (8 banks): 2x st [128,2,512] double-buffered (4), 2x O-acc
[65,512] (one head-pair live at a time), 2x qkv/outproj accumulators.
"""
import numpy as np

import concourse.bass as bass
import concourse.mybir as mybir
import concourse.tile as tile
from concourse import bacc
from concourse.bass_utils import run_bass_kernel_spmd

F32 = mybir.dt.float32
F32R = mybir.dt.float32r
BF16 = mybir.dt.bfloat16
AF = mybir.ActivationFunctionType

D_MODEL = 1024
N_HEADS = 16
HD = 64
B = 2
L = 2048                      # tokens per batch
HPC = 4                       # heads per core
DG = HPC * HD                 # 256 dims per core's head group
QB = 512                      # q-block width
N_QB = L // QB                # 4
N_DC = D_MODEL // 128         # 8 d_model chunks
N_TT = L // 128               # 16 token tiles

# q-start offset per diagonal chunk position (c_rel 0..3); N = QB - q0 must
# stay >= 256 for full fp32r rate, so c_rel=3 pads down to q0=256.
_Q0 = (0, 128, 256, 384)


def _patch_act_tables():
    """Make Exp and Ln resolve to the one table set containing both, so the
    per-qblock reciprocal (exp(-ln(x))) never triggers a ~2.7us table switch."""
    from concourse.hw_specs import get_activation_tables
    tabs = get_activation_tables("gen3")
    combined = "natural_log_exp_and_others"
    if combined in tabs:
        for name, fns in tabs.items():
            if name != combined:
                fns.discard(AF.Exp)
                fns.discard(AF.Ln)


def _build():
    _patch_act_tables()
    nc = bacc.Bacc("TRN2", target_bir_lowering=False)

    # x^T in quad-chunk layout [qb][quad][p][ci][q]: each DMA moves a
    # [128, 4, 512] quad with 4KB contiguous per partition (large packets)
    xt = nc.dram_tensor("xt", [N_QB, 2, 128, 4, QB], BF16,
                        kind="ExternalInput")
    wq = nc.dram_tensor("wq", [128, N_DC, DG], BF16, kind="ExternalInput")
    wk = nc.dram_tensor("wk", [128, N_DC, DG], BF16, kind="ExternalInput")
    wv = nc.dram_tensor("wv", [128, N_DC, DG], BF16, kind="ExternalInput")
    wo = nc.dram_tensor("wo", [128, 2, D_MODEL], BF16, kind="ExternalInput")
    bq = nc.dram_tensor("bq", [128, 2], F32, kind="ExternalInput")
    bk = nc.dram_tensor("bk", [128, 2], F32, kind="ExternalInput")
    bv = nc.dram_tensor("bv", [1, DG], BF16, kind="ExternalInput")
    # mask[p, j] = 1 iff j >= 128 + p.  mask[:, 128:] is a [128,128] tril
    # (applied to the 128-q strip containing the diagonal of chunks c_rel
    # 0..2); the full [128,256] handles c_rel=3 whose computed range starts
    # 128 q before its diagonal strip.
    mask = nc.dram_tensor("mask", [128, 128], BF16, kind="ExternalInput")
    # partials summed on host in fp32; bf16 partials cost ~1e-3 extra rel
    # err and halve the 8MB-per-core output traffic (and the tail DMA)
    out = nc.dram_tensor("out", [L, D_MODEL], BF16, kind="ExternalOutput")

    with tile.TileContext(nc) as tc:
        with (
            tc.tile_pool(name="cst", bufs=1) as cst,
            tc.tile_pool(name="xtp", bufs=2) as xtp,
            tc.tile_pool(name="ptp", bufs=5) as ptp,
            tc.tile_pool(name="nrm", bufs=2) as nrm,
            tc.tile_pool(name="osb", bufs=3) as osb,
            tc.tile_pool(name="ps_st", bufs=2, space="PSUM") as ps_st,
            tc.tile_pool(name="ps_ot", bufs=1, space="PSUM") as ps_ot,
            tc.tile_pool(name="ps_acc", bufs=2, space="PSUM") as ps_acc,
        ):
            # ---- constants / weights ----
            wq_sb = cst.tile([128, N_DC, DG], BF16, tag="wq")
            wk_sb = cst.tile([128, N_DC, DG], BF16, tag="wk")
            wv_sb = cst.tile([128, N_DC, DG], BF16, tag="wv")
            wo_sb = cst.tile([128, 2, D_MODEL], BF16, tag="wo")
            bq_sb = cst.tile([128, 2], F32, tag="bq")
            bk_sb = cst.tile([128, 2], F32, tag="bk")
            bv_sb = cst.tile([1, DG], BF16, tag="bv")
            mask_sb = cst.tile([128, 128], BF16, tag="mask")

            # ---- resident activation tensors ----
            qt_sb = [cst.tile([128, L], BF16, tag=f"qt{t}", name=f"qt{t}")
                     for t in range(2)]
            kt_sb = [cst.tile([128, L], BF16, tag=f"kt{t}", name=f"kt{t}")
                     for t in range(2)]
            ot_sb = [cst.tile([128, L], BF16, tag=f"ot{t}", name=f"ot{t}")
                     for t in range(2)]
            v_sb = [cst.tile([128, HPC, HD + 1], BF16, tag=f"v{tt}", name=f"v{tt}")
                    for tt in range(N_TT)]

            # ---- startup DMA: x block 0 + weights over THREE queues.
            # The SP queue (sync) comes alive first (~8.8us) — it carries the
            # first-matmul gate (wq chunk + x chunk). The SWDGE queue
            # (gpsimd) spins up ~3us later; the ACT ring (scalar) is free
            # until the first exp at ~25us. Order follows consumption:
            # q-units -> v-units -> k-units. ----
            xt0 = xtp.tile([128, N_DC, QB], BF16, tag="xt", name="xt0")
            nc.sync.dma_start(wq_sb[:, 0:4], wq[:, 0:4, :])
            nc.scalar.dma_start(xt0[:, 0:2, :], xt[0, 0][:, 0:2, :])
            nc.gpsimd.dma_start(wq_sb[:, 4:], wq[:, 4:, :])
            nc.scalar.dma_start(xt0[:, 2:4, :], xt[0, 0][:, 2:4, :])
            nc.sync.dma_start(xt0[:, 4:, :], xt[0, 1])
            nc.scalar.dma_start(bq_sb, bq[:, :])
            nc.sync.dma_start(wk_sb[:, 0:4], wk[:, 0:4, :])
            nc.scalar.dma_start(wk_sb[:, 4:], wk[:, 4:, :])
            nc.gpsimd.dma_start(wv_sb, wv[:, :, :])
            nc.gpsimd.dma_start(bv_sb, bv[:, :])
            nc.sync.dma_start(bk_sb, bk[:, :])
            nc.gpsimd.dma_start(mask_sb, mask[:, :])

            ones_f = cst.tile([128, HPC], F32, tag="ones_f")
            nc.vector.memset(ones_f, 1.0)
            ones1_f = cst.tile([1, 128], F32, tag="ones1_f")
            nc.vector.memset(ones1_f, 1.0)
            ones1 = cst.tile([1, 128], F32R, tag="ones1")
            nc.vector.tensor_copy(ones1, ones1_f)
            ones1b = cst.tile([1, 128], BF16, tag="ones1b")
            nc.vector.tensor_copy(ones1b, ones1_f)
            # touch Exp right away: the ~2.7us ACT table load lands here at
            # t~1us instead of stalling the first attention step at ~27us
            warm = cst.tile([1, 16], F32, tag="warm")
            nc.scalar.activation(warm, ones1_f[:, 0:16], AF.Exp)
            # keep the PE's HAM clock gate warm while the startup DMAs
            # stream: a chain of dummy matmuls covers t~1.3us until the
            # first real matmul (~11.5us); without it qkv(0) runs its first
            # ~9us at the cold 1.2GHz clock. Results are never read — the
            # WAW rotation through the acc pool just serializes the chain.
            for _ in range(28):
                dmy = ps_acc.tile([128, 128], F32, tag="mm", name="dmy")
                nc.tensor.matmul(dmy, ones1, ones1, start=True, stop=True)

            def load_xt(tb, queues=("sync", "gpsimd")):
                xt_t = xtp.tile([128, N_DC, QB], BF16, tag="xt", name="xt_t")
                for qd in range(2):
                    q = getattr(nc, queues[qd])
                    q.dma_start(xt_t[:, 4 * qd:4 * (qd + 1), :], xt[tb, qd])
                return xt_t

            def qkv_units(tb, xt_t):
                """QKV projection for token block tb: fine sub-units (~4 mm
                each) so the interleaver can pack them between attn steps.
                Order q, v, k matches the startup weight-DMA order."""
                units = []

                def qk_unit(t, w_sb, b_sb, dst):
                    hold = {}

                    def f1():
                        acc = ps_acc.tile([128, QB], F32, tag="mm", name="acc")
                        hold["acc"] = acc
                        for c in range(N_DC // 2):
                            nc.tensor.matmul(
                                acc,
                                w_sb[:, c, 128 * t:128 * (t + 1)],
                                xt_t[:, c, :],
                                start=(c == 0), stop=False,
                            )

                    def f2():
                        acc = hold["acc"]
                        for c in range(N_DC // 2, N_DC):
                            nc.tensor.matmul(
                                acc,
                                w_sb[:, c, 128 * t:128 * (t + 1)],
                                xt_t[:, c, :],
                                start=False, stop=(c == N_DC - 1),
                            )
                        nc.vector.tensor_scalar_add(
                            dst[t][:, tb * QB:(tb + 1) * QB], acc,
                            b_sb[:, t:t + 1],
                        )
                    return [f1, f2]

                def v_unit(j):
                    hold = {}
                    tt = tb * (QB // 128) + j

                    def f1():
                        vps = ps_acc.tile([128, DG], F32, tag="mm", name="vps")
                        hold["acc"] = vps
                        for c in range(N_DC // 2):
                            nc.tensor.matmul(
                                vps,
                                xt_t[:, c, j * 128:(j + 1) * 128],
                                wv_sb[:, c, :],
                                start=(c == 0), stop=False,
                            )

                    def f2():
                        vps = hold["acc"]
                        for c in range(N_DC // 2, N_DC):
                            nc.tensor.matmul(
                                vps,
                                xt_t[:, c, j * 128:(j + 1) * 128],
                                wv_sb[:, c, :],
                                start=False, stop=False,
                            )
                        nc.tensor.matmul(vps, ones1b, bv_sb, start=False,
                                         stop=True)
                        nc.vector.tensor_copy(
                            v_sb[tt][:, :, 0:HD],
                            vps.rearrange("p (h d) -> p h d", h=HPC),
                        )
                        nc.vector.tensor_copy(v_sb[tt][:, :, HD], ones_f)
                    return [f1, f2]

                for t in range(2):
                    units += qk_unit(t, wq_sb, bq_sb, qt_sb)
                kv = []
                for j in range(QB // 128):
                    kv += v_unit(j)
                for t in range(2):
                    kv += qk_unit(t, wk_sb, bk_sb, kt_sb)
                # (q_units, kv_units): attention on block tb needs the
                # q_units for every step, but kv_units only from the step
                # that touches this block's k-chunks — kv can lag.
                return units, kv



            def attn_steps(qb, final_units=(), free_units=()):
                """Attention for q-block qb as pipelined steps.

                Step i emits [O(i-2), norm-if-pair-done, S(i), exp(i)]: the
                O-accumulate lags S by TWO steps, so its exp has two full
                steps (plus interleaved filler) of ACT time — the PE never
                waits on the co-bottlenecked ACT engine.

                final_units (outproj of this qb) are interleaved into the
                last norm at half-block granularity to keep the PE warm
                through the tail.
                """
                n_kc = 4 * (qb + 1)
                seq = [(t, kc) for t in range(2) for kc in range(n_kc)]
                p_hold = {}
                ot_state = {}

                def q0_of(kc):
                    c_rel = kc - 4 * qb
                    return 0 if c_rel < 0 else _Q0[c_rel]

                def emit_s(t, kc):
                    q0 = q0_of(kc)
                    c_rel = kc - 4 * qb
                    # both head-halves packed in one 2-bank tile so a single
                    # exp covers them (halves ACT instruction overhead)
                    st = ps_st.tile([128, 2, QB], F32, tag="st", name="st")
                    p = ptp.tile([128, 2, QB], BF16, tag="pt", name="pt")
                    for hp in range(2):
                        nc.tensor.matmul(
                            st[:, hp, q0:],
                            kt_sb[t][64 * hp:64 * (hp + 1),
                                     kc * 128:(kc + 1) * 128],
                            qt_sb[t][64 * hp:64 * (hp + 1),
                                     qb * QB + q0:(qb + 1) * QB],
                            start=True, stop=True,
                        )
                    nc.scalar.activation(p[:, :, q0:], st[:, :, q0:], AF.Exp)
                    if c_rel >= 0:
                        for hp in range(2):
                            nc.vector.tensor_mul(
                                p[:, hp, q0:q0 + 128],
                                p[:, hp, q0:q0 + 128],
                                mask_sb)
                    p_hold[(t, kc)] = p

                def emit_o(t, kc):
                    if kc == 0:
                        ot_state[t] = [
                            ps_ot.tile([HD + 1, QB], F32, tag=f"otp{hp}",
                                       name=f"otp{hp}") for hp in range(2)]
                    ot_p = ot_state[t]
                    q0 = q0_of(kc)
                    p = p_hold.pop((t, kc))
                    for hp in range(2):
                        nc.tensor.matmul(
                            ot_p[hp][:, q0:],
                            v_sb[kc][:, 2 * t + hp, 0:HD + 1],
                            p[:, hp, q0:],
                            start=(kc == 0), stop=(kc == n_kc - 1),
                        )

                def emit_norm(t):
                    # 1/rowsum = exp(-ln(rowsum)) on ACT (same table set, no
                    # switch); partition-broadcast on the idle GPSIMD engine,
                    # so the final mul needs only one PSUM operand (ot_p).
                    ot_p = ot_state[t]
                    for hp in range(2):
                        lnrs = nrm.tile([1, QB], F32, tag="lnrs",
                                        name="lnrs")
                        nc.scalar.activation(lnrs, ot_p[hp][HD:HD + 1, :],
                                             AF.Ln)
                        recip = nrm.tile([1, QB], F32R, tag="recip",
                                         name="recip")
                        nc.scalar.activation(recip, lnrs, AF.Exp,
                                             scale=-1.0)
                        rbc = nrm.tile([HD, QB], F32R, tag="rbc",
                                       name="rbc")
                        nc.gpsimd.partition_broadcast(rbc, recip)
                        nc.vector.tensor_mul(
                            ot_sb[t][64 * hp:64 * (hp + 1),
                                     qb * QB:(qb + 1) * QB],
                            ot_p[hp][0:HD, :],
                            rbc,
                        )

                def emit_norm_half(t, h):
                    # column-half recip chain: ln -> exp(-x) -> partition
                    # broadcast -> multiply, for q columns [256h, 256h+256)
                    ot_p = ot_state[t]
                    cs = slice(256 * h, 256 * (h + 1))
                    recip = nrm.tile([1, 2, 256], F32R, tag="recip2",
                                     name="recip2")
                    for hp in range(2):
                        lnrs = nrm.tile([1, 256], F32, tag="lnrs",
                                        name="lnrs")
                        nc.scalar.activation(lnrs, ot_p[hp][HD:HD + 1, cs],
                                             AF.Ln)
                        nc.scalar.activation(recip[0:1, hp, :], lnrs,
                                             AF.Exp, scale=-1.0)
                    rbc = nrm.tile([HD, 2, 256], F32R, tag="rbc2",
                                   name="rbc2")
                    nc.gpsimd.partition_broadcast(rbc, recip)
                    for hp in range(2):
                        nc.vector.tensor_mul(
                            ot_sb[t][64 * hp:64 * (hp + 1),
                                     qb * QB + 256 * h:
                                     qb * QB + 256 * (h + 1)],
                            ot_p[hp][0:HD, cs],
                            rbc[:, hp, :],
                        )

                fin = list(final_units)
                nh = len(fin) // 2

                def make_step(i):
                    def f():
                        if i >= 2:
                            pt, pkc = seq[i - 2]
                            emit_o(pt, pkc)
                            if fin and pt == 1:
                                # final q-block, pair 1: the diagonal chunks
                                # n_kc-2/n_kc-1 only touch q >= 256, so the
                                # first column-half of the norm (and the
                                # outproj tiles it feeds) overlaps the last
                                # two attention steps
                                if pkc == n_kc - 3:
                                    for u in free_units:
                                        u()
                                    emit_norm_half(1, 0)
                                    for u in fin[:nh]:
                                        u()
                                elif pkc == n_kc - 1:
                                    emit_norm_half(1, 1)
                                    for u in fin[nh:]:
                                        u()
                            elif pkc == n_kc - 1:
                                emit_norm(pt)
                        if i < len(seq):
                            emit_s(*seq[i])
                    return f

                return [make_step(i) for i in range(len(seq) + 2)]

            def outproj_units(qb):
                """Output projection for q-block qb: 8 sub-units of
                [2 matmul + copy]; each token tile assembles a [128,1024]
                buffer so the out-DMA moves 4KB per partition (large DMA
                packets), queues alternating per tile."""
                units = []

                def op_unit(j, dc, hold):
                    def f():
                        tt = qb * (QB // 128) + j
                        ops = ps_acc.tile([128, 512], F32, tag="mm",
                                          name="ops")
                        for t in range(2):
                            nc.tensor.matmul(
                                ops,
                                ot_sb[t][:, tt * 128:(tt + 1) * 128],
                                wo_sb[:, t, dc * 512:(dc + 1) * 512],
                                start=(t == 0), stop=(t == 1),
                            )
                        if dc == 0:
                            hold["ob"] = osb.tile([128, D_MODEL], BF16,
                                                  tag="ob", name="ob")
                        ob = hold["ob"]
                        nc.vector.tensor_copy(
                            ob[:, dc * 512:(dc + 1) * 512], ops)
                        if dc == 1:
                            # the last q-block's final two tiles go on the
                            # SP queue: the end-of-kernel SWDGE drain is
                            # slow, so gpsimd's last out-DMA must be early
                            even = (j % 2 == 0) if qb < 3 else (j >= 2)
                            q = nc.sync if even else nc.gpsimd
                            q.dma_start(out[tt * 128:(tt + 1) * 128, :], ob)
                    return f

                for j in range(QB // 128):
                    hold = {}
                    for dc in range(2):
                        units.append(op_unit(j, dc, hold))
                return units

            def emit_interleaved(a_units, b_units):
                """Merge two unit lists proportionally (a is the primary
                stream); keeps relative order within each list."""
                na, nb = len(a_units), len(b_units)
                if nb == 0:
                    for u in a_units:
                        u()
                    return
                bi = 0
                for ai, u in enumerate(a_units):
                    u()
                    want = ((ai + 1) * nb) // na
                    while bi < want:
                        b_units[bi]()
                        bi += 1
                while bi < nb:
                    b_units[bi]()
                    bi += 1

            def emit_front_loaded(a_units, front_units, b_units, n_front):
                """front_units distributed over the first n_front steps (they
                must complete by then); b_units spread over the rest."""
                nf = len(front_units)
                fi = 0
                for i, u in enumerate(a_units[:n_front]):
                    u()
                    want = ((i + 1) * nf) // n_front
                    while fi < want:
                        front_units[fi]()
                        fi += 1
                while fi < nf:
                    front_units[fi]()
                    fi += 1
                emit_interleaved(a_units[n_front:], b_units)

            # ---- emission ----
            # Global PE and ACT(exp) load are nearly equal, so filler
            # distribution decides the wall-clock: every attention window
            # must carry enough matmul filler to cover its exp time, else
            # the PE idles and HAM re-throttles it to half clock.
            # KV-lazy: block tb's k/v projections ride INSIDE attention
            # windows (they are only needed once the diagonal approaches),
            # freeing earlier windows' filler for where it is scarce.
            xt1 = load_xt(1, queues=("gpsimd", "sync"))
            nc.gpsimd.dma_start(wo_sb, wo[:, :, :])
            q0u, kv0u = qkv_units(0, xt0)
            for u in q0u + kv0u:
                u()
            q1u, kv1u = qkv_units(1, xt1)
            emit_interleaved(attn_steps(0), q1u + kv1u)
            xt2 = load_xt(2)
            q2u, kv2u = qkv_units(2, xt2)
            emit_interleaved(attn_steps(1), q2u + kv2u[:6])
            xt3 = load_xt(3, queues=("gpsimd", "sync"))
            q3u, kv3u = qkv_units(3, xt3)
            op012 = outproj_units(0) + outproj_units(1) + outproj_units(2)
            emit_front_loaded(attn_steps(2), kv2u[6:], q3u + op012[:2],
                              n_front=8)
            emit_front_loaded(
                attn_steps(3, final_units=outproj_units(3),
                           free_units=op012[-4:]),
                kv3u, op012[2:-4], n_front=12)

    nc.compile()
    return nc


_NC_CACHE = None


def _get_nc():
    global _NC_CACHE
    if _NC_CACHE is None:
        _NC_CACHE = _build()
    return _NC_CACHE


import ml_dtypes

BF16_NP = ml_dtypes.bfloat16


def _sw_w(w):
    """[C*128, M] -> [128, C, M] (SBUF layout, contiguous per partition)."""
    c128, m = w.shape
    return np.ascontiguousarray(
        w.reshape(c128 // 128, 128, m).transpose(1, 0, 2).astype(BF16_NP))


def _make_mask():
    p_ = np.arange(128)[:, None]
    j_ = np.arange(128)[None, :]
    return np.ascontiguousarray((j_ >= p_).astype(BF16_NP))


def kernel(x, qkv_w, qkv_b, out_w, out_b, _trace=False):
    x = np.asarray(x, dtype=np.float32)
    qkv_w = np.asarray(qkv_w, dtype=np.float32)
    qkv_b = np.asarray(qkv_b, dtype=np.float32)
    out_w = np.asarray(out_w, dtype=np.float32)
    out_b = np.asarray(out_b, dtype=np.float32)

    scale = 1.0 / np.sqrt(HD)
    wq_full = qkv_w[0:D_MODEL] * scale          # [1024, 1024]
    wk_full = qkv_w[D_MODEL:2 * D_MODEL]
    wv_full = qkv_w[2 * D_MODEL:3 * D_MODEL]
    bq_full = qkv_b[0:D_MODEL] * scale
    bk_full = qkv_b[D_MODEL:2 * D_MODEL]
    bv_full = qkv_b[2 * D_MODEL:3 * D_MODEL]

    mask = _make_mask()
    in_maps = []
    for core in range(8):
        b, g = core // 4, core % 4
        sl = slice(DG * g, DG * (g + 1))
        # xt: x[b]^T [1024, 2048] -> [qb, quad, p, ci, q] = [4, 2, 128, 4,
        # 512]; each quad is 4KB contiguous per partition (large packets)
        xt_sw = np.ascontiguousarray(
            x[b].T.reshape(2, 4, 128, N_QB, QB).transpose(3, 0, 2, 1, 4)
            .astype(BF16_NP))
        in_maps.append({
            "xt": xt_sw,
            "wq": _sw_w(wq_full[sl].T),
            "wk": _sw_w(wk_full[sl].T),
            "wv": _sw_w(wv_full[sl].T),
            "wo": _sw_w(out_w[:, sl].T),
            "bq": np.ascontiguousarray(bq_full[sl].reshape(2, 128).T),
            "bk": np.ascontiguousarray(bk_full[sl].reshape(2, 128).T),
            "bv": np.ascontiguousarray(bv_full[sl].reshape(1, DG).astype(BF16_NP)),
            "mask": mask,
        })

    nc = _get_nc()
    res = run_bass_kernel_spmd(nc, in_maps, core_ids=list(range(8)),
                               trace=_trace)

    final = np.zeros((B, L, D_MODEL), dtype=np.float32)
    for core in range(8):
        b = core // 4
        final[b] += res.results[core]["out"].astype(np.float32)
    final += out_b[None, None, :]

    kernel.last_results = res
    return final
